# revision 19
# baseline (speedup 1.0000x reference)
#!/usr/bin/env python3
"""Trainium2 Bass kernel for nn_DecoderBlock (B=4,T=1024,C=1024,H=16,FFN=C).

Sharding: 8 NeuronCores, zero collectives. Core c owns 512 query tokens of
batch b=c//2 (half h=c%2) and computes the whole decoder block for them
end-to-end: causal self-attention over its batch's 1024 keys (full rectangle
+ additive mask; the host permutes tokens so the owned 512 always sit first,
giving a single SPMD program), cross-attention over the encoder, FFN. The
host splits inputs and reassembles the output.

On-device: natural-layout residual stream [tokens, C] in fp32. LayerNorm
stats+apply run in natural layout (per-partition scalars); the [C, tokens]
operand every projection needs is produced by DMA-engine transposes (bf16),
keeping the PE free. Matmul path is bf16 with fp32 PSUM accumulation and an
fp32 residual stream. Wide-K projection chains are split into two K=64
row-group halves running concurrently on the PE (hides LDWEIGHTS + pipe
drain). Attention runs in S^T layout [tk, tq]: the two heads of a pair are
row-group packed K=64 matmuls; exp runs on the scalar engine straight out
of PSUM (scale=1/8 fused); the causal mask is added into PSUM via an
identity-matmul; softmax sums come from an appended ones-column in V (M=65
matmuls); the division is an approx-reciprocal + gpsimd partition-broadcast
fused multiply. LN gamma/beta are folded into the weights on the host;
projection biases are all zero in this problem (verified at runtime) and
skipped, with a rank-1 matmul fallback otherwise.
"""
import sys
if "/opt/trn_rl_repo" not in sys.path:
    sys.path.insert(0, "/opt/trn_rl_repo")

import numpy as np
import ml_dtypes

import concourse.bass as bass
import concourse.mybir as mybir
import concourse.tile as tile
from concourse import bacc
from concourse import bass_utils

B, T, TE, C, H, HD = 4, 1024, 1024, 1024, 16, 64
NCORES = 8
TOWN = T // 2
EPS = 1e-5
F32 = mybir.dt.float32
BF16 = mybir.dt.bfloat16
AF = mybir.ActivationFunctionType
ALU = mybir.AluOpType
AXX = mybir.AxisListType.X
NT_KV = T // 128
NT_OWN = TOWN // 128
NC_T = C // 128
NHP = H // 2
BF = np.dtype(ml_dtypes.bfloat16)
import os
USE_DMA_T = os.environ.get("USE_DMA_T", "0") == "1"
USE_KSPLIT = os.environ.get("USE_KSPLIT", "0") == "1"


def _ksplit_chain(nc, ps, lhs_tiles, rhs_tiles, n_k, bias_mm=None):
    """Accumulate sum_k lhs[k].T @ rhs[k] into psum `ps` as two concurrent
    K=64 row-group chains. lhs_tiles/rhs_tiles: callables k -> AP."""
    if not USE_KSPLIT:
        for k in range(n_k):
            last = (k == n_k - 1) and bias_mm is None
            nc.tensor.matmul(ps, lhs_tiles(k)[:], rhs_tiles(k)[:],
                             start=(k == 0), stop=last, skip_group_check=True)
        if bias_mm is not None:
            bias_mm()
        return
    for k in range(n_k):
        l = lhs_tiles(k); r = rhs_tiles(k)
        nc.tensor.matmul(ps, l[0:64, :], r[0:64, :], start=(k == 0), stop=False,
                         skip_group_check=True)
    for k in range(n_k):
        l = lhs_tiles(k); r = rhs_tiles(k)
        last = (k == n_k - 1) and bias_mm is None
        nc.tensor.matmul(ps, l[64:128, :], r[64:128, :], start=False, stop=last,
                         skip_group_check=True)
    if bias_mm is not None:
        bias_mm()


def _layer_norm_T(nc, sb, stats, x_tiles, n_t, pfx, eps_ap):
    """LN over n_t natural [128, C] tiles -> NC_T transposed bf16 tiles
    [128, n_t*128] (DMA-engine transposes)."""
    msum = stats.tile([128, n_t], F32, tag="msum", bufs=2, name=f"msum{pfx}")
    m_neg = stats.tile([128, n_t], F32, tag="mneg", bufs=2, name=f"mneg{pfx}")
    var_raw = stats.tile([128, n_t], F32, tag="vraw", bufs=2, name=f"vraw{pfx}")
    lnv = stats.tile([128, n_t], F32, tag="lnv", bufs=2, name=f"lnv{pfx}")
    rstd = stats.tile([128, n_t], F32, tag="rstd", bufs=2, name=f"rstd{pfx}")

    # Per-tile vector stats (pipeline with the producing projection); the
    # scalar activations stay phase-grouped — interleaving Square/Ln/Exp
    # per tile thrashes the activation table (~1.3us per reload).
    for i in range(n_t):
        nc.vector.reduce_sum(msum[:, i:i + 1], x_tiles[i][:], axis=AXX)
        nc.vector.tensor_scalar_mul(m_neg[:, i:i + 1], msum[:, i:i + 1],
                                    -1.0 / C)
    for i in range(n_t):
        scr = sb.tile([128, C], BF16, tag="sqscr", bufs=1, name=f"scr{pfx}{i}")
        nc.scalar.activation(scr[:], x_tiles[i][:], AF.Square,
                             bias=m_neg[:, i:i + 1], accum_out=var_raw[:, i:i + 1])
    nc.scalar.activation(lnv[:], var_raw[:], AF.Ln, scale=1.0 / C, bias=eps_ap)
    nc.scalar.activation(rstd[:], lnv[:], AF.Exp, scale=-0.5)

    out = [sb.tile([128, n_t * 128], BF16, tag=f"lnT{n_t}", bufs=(9 if n_t == NT_KV else NC_T),
                   name=f"lnT{pfx}{ci}") for ci in range(NC_T)]
    with tc_ref[0].tile_pool(name=f"lnp{pfx}", bufs=4, space="PSUM") as lnp:
        for g in range(0, n_t, 4):
            gn = min(4, n_t - g)
            ln_nat = []
            for i in range(g, g + gn):
                t = sb.tile([128, C], BF16, tag="lnnat", bufs=4, name=f"nat{pfx}{i}")
                nc.vector.tensor_scalar(t[:], x_tiles[i][:], m_neg[:, i:i + 1],
                                        rstd[:, i:i + 1], op0=ALU.add, op1=ALU.mult)
                ln_nat.append(t)
            for ci in range(NC_T):
                ps = lnp.tile([128, 512], BF16, tag="lnp", name=f"lnps{pfx}{ci}{g}")
                for j in range(gn):
                    nc.tensor.transpose(ps[:, j * 128:(j + 1) * 128],
                                        ln_nat[j][:, ci * 128:(ci + 1) * 128],
                                        ident_ref[0][:])
                    nc.vector.tensor_copy(out[ci][:, (g + j) * 128:(g + j + 1) * 128],
                                          ps[:, j * 128:(j + 1) * 128])
    return out


tc_ref = [None]
ident_ref = [None]


def _load_w512(nc, wpool, dram_ap, pfx):
    tiles = []
    for ci in range(NC_T):
        row = []
        for nch in range(2):
            t = wpool.tile([128, 512], BF16, tag="w512", bufs=16,
                           name=f"w{pfx}{ci}_{nch}")
            nc.sync.dma_start(t[:], dram_ap[ci * 128:(ci + 1) * 128,
                                            nch * 512:(nch + 1) * 512])
            row.append(t)
        tiles.append(row)
    return tiles


GROUPS = [[2 * i, 2 * i + 1] for i in range(4)]


def _k_phase(nc, tc, sb, wpool, src_kvT, w_k, b_k, ones_row, k_own_d, tag):
    """K^T own-half columns for all NHP head-pairs; write them to the
    exchange scratch. Returns kT tiles [128, T] with cols [TOWN:T] filled."""
    kTs = []
    with tc.tile_pool(name=f"psk{tag}", bufs=2, space="PSUM") as ps_qk:
        for hp in range(NHP):
            wk_t = []
            for ci in range(NC_T):
                tk_ = wpool.tile([128, 128], BF16, tag="w128", bufs=16,
                                 name=f"wk{tag}{hp}_{ci}")
                nc.sync.dma_start(tk_[:], w_k[ci * 128:(ci + 1) * 128,
                                              hp * 128:(hp + 1) * 128])
                wk_t.append(tk_)
            kT = sb.tile([128, T], BF16, tag=f"kT{tag}", bufs=NHP,
                         name=f"kT{tag}{hp}")
            ps = ps_qk.tile([128, TOWN], F32, tag="qk", name=f"psk{tag}{hp}")
            bias_mm = None
            if b_k is not None:
                def bias_mm(ps=ps, hp=hp):
                    nc.tensor.matmul(ps[:], b_k[:, hp * 128:(hp + 1) * 128],
                                     ones_row[:], start=False, stop=True,
                                     skip_group_check=True)
            _ksplit_chain(nc, ps[:], lambda k: wk_t[k][:],
                          lambda k: src_kvT[k][:, TOWN:T], NC_T, bias_mm)
            nc.vector.tensor_copy(kT[:, TOWN:T], ps[:])
            nc.sync.dma_start(k_own_d[hp * 128:(hp + 1) * 128, :],
                              kT[:, TOWN:T])
            kTs.append(kT)
    return kTs


def _q_phase(nc, tc, sb, wpool, src_qT, w_q, b_q, ones_row, tag):
    qTs = []
    with tc.tile_pool(name=f"psq{tag}", bufs=2, space="PSUM") as ps_qk:
        for hp in range(NHP):
            wq_t = []
            for ci in range(NC_T):
                tq_ = wpool.tile([128, 128], BF16, tag="w128", bufs=16,
                                 name=f"wq{tag}{hp}_{ci}")
                nc.sync.dma_start(tq_[:], w_q[ci * 128:(ci + 1) * 128,
                                              hp * 128:(hp + 1) * 128])
                wq_t.append(tq_)
            qT = sb.tile([128, TOWN], BF16, tag="qT", bufs=NHP,
                         name=f"qT{tag}{hp}")
            ps = ps_qk.tile([128, TOWN], F32, tag="qk", name=f"psq{tag}{hp}")
            bias_mm = None
            if b_q is not None:
                def bias_mm(ps=ps, hp=hp):
                    nc.tensor.matmul(ps[:], b_q[:, hp * 128:(hp + 1) * 128],
                                     ones_row[:], start=False, stop=True,
                                     skip_group_check=True)
            _ksplit_chain(nc, ps[:], lambda k: wq_t[k][:],
                          lambda k: src_qT[k][:], NC_T, bias_mm)
            nc.vector.tensor_copy(qT[:], ps[:])
            qTs.append(qT)
    return qTs


def _v_phase(nc, tc, sb, wpool, src_kvT, w_v, b_v, vflag, ones16, ones_row,
             v_own_d, tag):
    """V natural [tk, d] with per-head ones column, own tiles only; write
    them (full 1040-wide rows) to the exchange scratch."""
    wv_t = _load_w512(nc, wpool, w_v, f"v{tag}")
    v_sb = [sb.tile([128, H * 65], BF16, tag="vsb", bufs=13,
                    name=f"v{tag}{i}") for i in range(NT_KV)]
    for i in range(NT_OWN, NT_KV):
        vt = v_sb[i]
        ones_dst = vt[:].rearrange("p (h c) -> p h c", c=65)[:, :, 64:65]
        ones_src = ones16[:].rearrange("p (h c) -> p h c", c=1)
        if vflag is not None:
            nc.vector.tensor_scalar_mul(ones_dst, ones_src, vflag[:, i:i + 1])
        else:
            nc.vector.tensor_copy(ones_dst, ones_src)
    with tc.tile_pool(name=f"psv{tag}", bufs=4, space="PSUM") as ps_v:
        for nch in range(2):
            for i in range(NT_OWN, NT_KV):
                ps = ps_v.tile([128, 512], F32, tag="vp", name=f"psv{tag}{nch}{i}")
                bias_mm = None
                if b_v is not None:
                    def bias_mm(ps=ps, nch=nch):
                        nc.tensor.matmul(ps[:], ones_row[:, 0:128],
                                         b_v[:, nch * 512:(nch + 1) * 512],
                                         start=False, stop=True,
                                         skip_group_check=True)
                _ksplit_chain(nc, ps[:],
                              lambda k, i=i: src_kvT[k][:, i * 128:(i + 1) * 128],
                              lambda k, nch=nch: wv_t[k][nch][:],
                              NC_T, bias_mm)
                dst = v_sb[i][:, nch * 520:(nch + 1) * 520].rearrange(
                    "p (h c) -> p h c", c=65)[:, :, 0:64]
                src_ps = ps[:].rearrange("p (h c) -> p h c", c=64)
                if vflag is not None:
                    nc.vector.tensor_scalar_mul(dst, src_ps, vflag[:, i:i + 1])
                else:
                    nc.vector.tensor_copy(dst, src_ps)
    for i in range(NT_OWN, NT_KV):
        nc.sync.dma_start(v_own_d[(i - NT_OWN) * 128:(i - NT_OWN + 1) * 128, :],
                          v_sb[i][:])
    return v_sb


def _gather(nc, own_d, gath_d):
    nc.gpsimd.collective_compute(
        "AllGather", ALU.bypass, replica_groups=GROUPS,
        ins=[own_d[:].opt()], outs=[gath_d[:].opt()])


def _readback_k(nc, kTs, k_gath_d, gidx_t):
    for hp in range(NHP):
        nc.gpsimd.indirect_dma_start(
            out=kTs[hp][:, 0:TOWN], out_offset=None, in_=k_gath_d[:],
            in_offset=bass.IndirectOffsetOnAxis(ap=gidx_t[:, hp:hp + 1], axis=0))


def _readback_v(nc, v_sb, v_gath_d, gidx_t, vflag):
    for i in range(NT_OWN):
        nc.gpsimd.indirect_dma_start(
            out=v_sb[i][:], out_offset=None, in_=v_gath_d[:],
            in_offset=bass.IndirectOffsetOnAxis(ap=gidx_t[:, 8 + i:9 + i],
                                                axis=0))
        if vflag is not None:
            nc.vector.tensor_scalar_mul(v_sb[i][:], v_sb[i][:],
                                        vflag[:, i:i + 1])


def _scores(nc, tc, sb, qTs, kTs, v_sb, nt_k, tri01, tag, mask_from):
    """Scores/softmax/AV for all head-pairs. mask_from=NT_OWN → causal on
    own-half tiles (emitted first so they don't wait on the exchange);
    mask_from=nt_k → no causality."""
    avT = []
    if mask_from < nt_k:
        order = list(range(mask_from, nt_k)) + list(range(mask_from))
    else:
        order = list(range(nt_k))
    with (
        tc.tile_pool(name=f"pssc{tag}", bufs=4, space="PSUM") as ps_sc,
        tc.tile_pool(name=f"psav{tag}", bufs=2, space="PSUM") as ps_av,
    ):
        for hp in range(NHP):
            qT = qTs[hp]
            kT = kTs[hp]
            at = sb.tile([128, TOWN], BF16, tag="avT", bufs=NHP,
                         name=f"avT{tag}{hp}")
            pav = [ps_av.tile([65, TOWN], F32, tag="av", name=f"psav{tag}{hp}{s}")
                   for s in range(2)]
            for pos, i in enumerate(order):
                psc = [ps_sc.tile([128, TOWN], F32, tag="sc",
                                  name=f"pssc{tag}{hp}{i}{s}") for s in range(2)]
                causal = i >= mask_from
                col0 = 128 * (i - mask_from) if causal else 0
                for sub in range(2):
                    nc.tensor.matmul(psc[sub][:, col0:TOWN],
                                     kT[sub * 64:(sub + 1) * 64,
                                        i * 128:(i + 1) * 128],
                                     qT[sub * 64:(sub + 1) * 64, col0:TOWN],
                                     start=True, stop=True,
                                     skip_group_check=True)
                for sub in range(2):
                    h = hp * 2 + sub
                    es = sb.tile([128, TOWN], BF16, tag="exp", bufs=4,
                                 name=f"es{tag}{h}{i}")
                    nc.scalar.activation(es[:, col0:TOWN], psc[sub][:, col0:TOWN],
                                         AF.Exp, scale=float(HD) ** -0.5)
                    if causal:
                        nc.vector.tensor_mul(es[:, col0:col0 + 128],
                                             es[:, col0:col0 + 128], tri01[:])
                    nc.tensor.matmul(pav[sub][:, col0:TOWN],
                                     v_sb[i][:, h * 65:h * 65 + 65],
                                     es[:, col0:TOWN], start=(pos == 0),
                                     stop=(pos == nt_k - 1),
                                     skip_group_check=True)
            for sub in range(2):
                h = hp * 2 + sub
                s_h = sb.tile([1, TOWN], F32, tag="sums", bufs=2, name=f"s{tag}{h}")
                nc.vector.tensor_copy(s_h[:], pav[sub][64:65, :])
                rec = sb.tile([1, TOWN], F32, tag="rec", bufs=2, name=f"rec{tag}{h}")
                nc.vector.reciprocal_approx_fast(rec[:], s_h[:])
                r_bc = sb.tile([64, TOWN], F32, tag="rbc", bufs=1, name=f"rb{tag}{h}")
                nc.gpsimd.partition_broadcast(r_bc[:], rec[:])
                nc.vector.tensor_mul(at[sub * 64:(sub + 1) * 64, :],
                                     pav[sub][0:64, :], r_bc[:])
            avT.append(at)
    return avT


def _proj_residual(nc, sb, wpool, ps_pool, lhsT_tiles, w_dram, b_row,
                   resid_tiles, out_tiles, ones_row, tag):
    """out[tq, c'] = lhsT.T @ W + bias + resid (fp32 out)."""
    w_t = _load_w512(nc, wpool, w_dram, f"p{tag}")
    for tqt in range(NT_OWN):
        for nch in range(2):
            ps = ps_pool.tile([128, 512], F32, tag="pr", name=f"pr{tag}{nch}{tqt}")
            bias_mm = None
            if b_row is not None:
                def bias_mm(ps=ps, nch=nch):
                    nc.tensor.matmul(ps[:], ones_row[:, 0:128],
                                     b_row[:, nch * 512:(nch + 1) * 512],
                                     start=False, stop=True,
                                     skip_group_check=True)
            _ksplit_chain(nc, ps[:],
                          lambda k, tqt=tqt: lhsT_tiles[k][:, tqt * 128:(tqt + 1) * 128],
                          lambda k, nch=nch: w_t[k][nch][:],
                          NC_T, bias_mm)
            nc.vector.tensor_add(out_tiles[tqt][:, nch * 512:(nch + 1) * 512],
                                 ps[:], resid_tiles[tqt][:, nch * 512:(nch + 1) * 512])


def build(with_bias=False):
    nc = bacc.Bacc("TRN2", target_bir_lowering=False, debug=False,
                   num_devices=NCORES)
    d_in = {}

    def din(name, shape, dt=BF16):
        d_in[name] = nc.dram_tensor(name, shape, dt, kind="ExternalInput").ap()
        return d_in[name]

    x_kv = din("x_kv", [T, C])
    xe = din("xe", [TE, C])
    tri01_d = din("tri01", [128, 128])
    vflag_d = din("vflag", [128, NT_KV], F32)
    ident = din("ident", [128, 128])
    ones16_d = din("ones16", [128, 16])
    gidx_d = din("gidx", [128, 12], mybir.dt.int32)
    for w in ["wq_sa", "wk_sa", "wv_sa", "wp_sa", "wq_ca", "wk_ca", "wv_ca",
              "wp_ca", "w1", "w2"]:
        din(w, [C, C])
    bias_names = ["bq_sa", "bk_sa", "bv_sa", "bp_sa", "bq_ca", "bk_ca",
                  "bv_ca", "bp_ca", "b1", "b2"]
    if with_bias:
        ones_row_d = din("ones_row", [1, 512])
        for b in bias_names:
            din(b, [1, C])
    out_d = nc.dram_tensor("out", [TOWN, C], BF16, kind="ExternalOutput").ap()

    with tile.TileContext(nc) as tc:
        with (
            tc.tile_pool(name="sb", bufs=1) as sb,
            tc.tile_pool(name="stats", bufs=1) as stats,
            tc.tile_pool(name="wpool", bufs=1) as wpool,
            tc.tile_pool(name="dramp", bufs=1, space="DRAM") as dramp,
        ):
            sa_k_own = dramp.tile([NHP * 128, TOWN], BF16, name="sakown")
            sa_k_gath = dramp.tile([2 * NHP * 128, TOWN], BF16, name="sakgath")
            sa_v_own = dramp.tile([NT_OWN * 128, H * 65], BF16, name="savown")
            sa_v_gath = dramp.tile([2 * NT_OWN * 128, H * 65], BF16,
                                   name="savgath")
            ca_k_own = dramp.tile([NHP * 128, TOWN], BF16, name="cakown")
            ca_k_gath = dramp.tile([2 * NHP * 128, TOWN], BF16, name="cakgath")
            ca_v_own = dramp.tile([NT_OWN * 128, H * 65], BF16, name="cavown")
            ca_v_gath = dramp.tile([2 * NT_OWN * 128, H * 65], BF16,
                                   name="cavgath")
            ident_t = sb.tile([128, 128], BF16, tag="ident", name="identt")
            nc.sync.dma_start(ident_t[:], ident)
            tc_ref[0] = tc
            ident_ref[0] = ident_t
            ones16 = sb.tile([128, 16], BF16, tag="ones16", name="ones16t")
            nc.sync.dma_start(ones16[:], ones16_d)
            eps_ap = sb.tile([128, 1], F32, tag="epsap", name="epst")
            nc.gpsimd.memset(eps_ap[:], EPS)
            if with_bias:
                ones_row = sb.tile([1, 512], BF16, tag="onesrow", name="onesrowt")
                nc.sync.dma_start(ones_row[:], ones_row_d)

                def brow(name):
                    t = sb.tile([1, C], BF16, tag="brow", bufs=4, name=f"br{name}")
                    nc.sync.dma_start(t[:], d_in[name])
                    return t
            else:
                ones_row = None
                brow = lambda name: None

            x_tiles = []
            for i in range(NT_KV):
                t = sb.tile([128, C], BF16, tag="xkv", bufs=12, name=f"x{i}")
                nc.sync.dma_start(t[:], x_kv[i * 128:(i + 1) * 128, :])
                x_tiles.append(t)
            tri01_t = sb.tile([128, 128], BF16, tag="mask", name="tri01t")
            nc.sync.dma_start(tri01_t[:], tri01_d)
            vflag = sb.tile([128, NT_KV], F32, tag="vflag", name="vflagt")
            nc.sync.dma_start(vflag[:], vflag_d)
            gidx_t = sb.tile([128, 12], mybir.dt.int32, tag="gidx",
                             name="gidxt")
            nc.sync.dma_start(gidx_t[:], gidx_d)

            # ---------------- SA K/Q/V (own half) + pair exchange ---------
            ln1T = _layer_norm_T(nc, sb, stats, x_tiles, NT_KV, "l1", eps_ap[:])
            ln1T_own = [t[:, TOWN:] for t in ln1T]  # own tokens = rows 512..1023
            sa_kT = _k_phase(nc, tc, sb, wpool, ln1T, d_in["wk_sa"],
                             brow("bk_sa"), ones_row, sa_k_own, "sa")
            sa_qT = _q_phase(nc, tc, sb, wpool, ln1T_own, d_in["wq_sa"],
                             brow("bq_sa"), ones_row, "sa")
            sa_vsb = _v_phase(nc, tc, sb, wpool, ln1T, d_in["wv_sa"],
                              brow("bv_sa"), vflag, ones16, ones_row,
                              sa_v_own, "sa")
            _gather(nc, sa_k_own, sa_k_gath)
            _gather(nc, sa_v_own, sa_v_gath)

            # encoder LN + CA K/V projections cover the SA gather latency
            xe_tiles = []
            for i in range(NT_KV):
                t = sb.tile([128, C], BF16, tag="xkv", bufs=12, name=f"xe{i}")
                nc.sync.dma_start(t[:], xe[i * 128:(i + 1) * 128, :])
                xe_tiles.append(t)
            ln2T = _layer_norm_T(nc, sb, stats, xe_tiles, NT_KV, "l2", eps_ap[:])
            ca_kT = _k_phase(nc, tc, sb, wpool, ln2T, d_in["wk_ca"],
                             brow("bk_ca"), ones_row, ca_k_own, "ca")
            ca_vsb = _v_phase(nc, tc, sb, wpool, ln2T, d_in["wv_ca"],
                              brow("bv_ca"), None, ones16, ones_row,
                              ca_v_own, "ca")
            _readback_k(nc, sa_kT, sa_k_gath, gidx_t)
            _readback_v(nc, sa_vsb, sa_v_gath, gidx_t, vflag)
            _gather(nc, ca_k_own, ca_k_gath)
            _gather(nc, ca_v_own, ca_v_gath)

            # ---------------- SA attention + projection ------------------
            avT = _scores(nc, tc, sb, sa_qT, sa_kT, sa_vsb, NT_KV, tri01_t,
                          "sa", mask_from=NT_OWN)
            x1 = [sb.tile([128, C], BF16, tag="res1", bufs=NT_OWN, name=f"x1_{i}")
                  for i in range(NT_OWN)]
            with tc.tile_pool(name="pspr1", bufs=4, space="PSUM") as ps_pr:
                _proj_residual(nc, sb, wpool, ps_pr, avT, d_in["wp_sa"],
                               brow("bp_sa"), x_tiles[NT_OWN:], x1,
                               ones_row, "sa")

            # ---------------- CA ----------------
            ln3T = _layer_norm_T(nc, sb, stats, x1, NT_OWN, "l3", eps_ap[:])
            ca_qT = _q_phase(nc, tc, sb, wpool, ln3T, d_in["wq_ca"],
                             brow("bq_ca"), ones_row, "ca")
            _readback_k(nc, ca_kT, ca_k_gath, gidx_t)
            _readback_v(nc, ca_vsb, ca_v_gath, gidx_t, None)
            avT2 = _scores(nc, tc, sb, ca_qT, ca_kT, ca_vsb, NT_KV, tri01_t,
                           "ca", mask_from=NT_KV)
            x2 = [sb.tile([128, C], BF16, tag="res2", bufs=NT_OWN, name=f"x2_{i}")
                  for i in range(NT_OWN)]
            with tc.tile_pool(name="pspr2", bufs=4, space="PSUM") as ps_pr:
                _proj_residual(nc, sb, wpool, ps_pr, avT2, d_in["wp_ca"],
                               brow("bp_ca"), x1, x2, ones_row, "ca")

            # ---------------- FFN ----------------
            ln4T = _layer_norm_T(nc, sb, stats, x2, NT_OWN, "l4", eps_ap[:])
            b1r = brow("b1")
            w1_t = _load_w512(nc, wpool, d_in["w1"], "w1")
            gT = []
            with tc.tile_pool(name="psh", bufs=4, space="PSUM") as ps_h:
                for hidt in range(NC_T):
                    ps = ps_h.tile([128, TOWN], F32, tag="h", name=f"psh{hidt}")
                    bias_mm = None
                    if b1r is not None:
                        def bias_mm(ps=ps, hidt=hidt):
                            nc.tensor.matmul(
                                ps[:], b1r[:, hidt * 128:(hidt + 1) * 128],
                                ones_row[:], start=False, stop=True,
                                skip_group_check=True)
                    _ksplit_chain(
                        nc, ps[:],
                        lambda k, hidt=hidt: w1_t[k][hidt // 4][
                            :, (hidt % 4) * 128:(hidt % 4 + 1) * 128],
                        lambda k: ln4T[k][:], NC_T, bias_mm)
                    g = sb.tile([128, TOWN], BF16, tag="gT", bufs=NC_T,
                                name=f"g{hidt}")
                    nc.scalar.activation(g[:], ps[:], AF.Gelu)
                    gT.append(g)
            out_sb = [sb.tile([128, C], BF16, tag="res1", bufs=NT_OWN,
                              name=f"osb{i}") for i in range(NT_OWN)]
            with tc.tile_pool(name="psf", bufs=4, space="PSUM") as ps_f:
                _proj_residual(nc, sb, wpool, ps_f, gT, d_in["w2"],
                               brow("b2"), x2, out_sb, ones_row, "f")
            for tqt in range(NT_OWN):
                nc.sync.dma_start(out_d[tqt * 128:(tqt + 1) * 128, :],
                                  out_sb[tqt][:])
    nc.compile()
    return nc


_CACHED = {}


def _get_nc(with_bias):
    if with_bias not in _CACHED:
        _CACHED[with_bias] = build(with_bias)
    return _CACHED[with_bias]


def _stack_heads(w):
    return np.ascontiguousarray(np.transpose(np.asarray(w), (1, 0, 2))
                                .reshape(C, H * HD))


def prepare_in_maps(inputs):
    inp = {k: np.asarray(v, dtype=np.float32) for k, v in inputs.items()}
    g1, be1 = inp["g1"], inp["be1"]
    g2, be2 = inp["g2"], inp["be2"]
    g3, be3 = inp["g3"], inp["be3"]
    g4, be4 = inp["g4"], inp["be4"]

    wq_sa = _stack_heads(inp["Wq_sa"]); wk_sa = _stack_heads(inp["Wk_sa"])
    wv_sa = _stack_heads(inp["Wv_sa"])
    wq_ca = _stack_heads(inp["Wq_ca"]); wk_ca = _stack_heads(inp["Wk_ca"])
    wv_ca = _stack_heads(inp["Wv_ca"])

    biases = {
        "bq_sa": be1 @ wq_sa, "bk_sa": be1 @ wk_sa, "bv_sa": be1 @ wv_sa,
        "bp_sa": inp["bp_sa"],
        "bq_ca": be3 @ wq_ca, "bk_ca": be2 @ wk_ca, "bv_ca": be2 @ wv_ca,
        "bp_ca": inp["bp_ca"],
        "b1": inp["b1"] + be4 @ inp["W1"], "b2": inp["b2"],
    }
    with_bias = any(np.abs(v).max() > 0 for v in biases.values())

    tl128 = np.arange(128)
    shared = {
        "ident": np.eye(128, dtype=np.float32),
        "ones16": np.ones((128, 16), np.float32),
        "tri01": np.where(tl128[:, None] <= tl128[None, :], 1.0, 0.0),
        "wq_sa": g1[:, None] * wq_sa,
        "wk_sa": g1[:, None] * wk_sa,
        "wv_sa": g1[:, None] * wv_sa,
        "wp_sa": inp["Wp_sa"],
        "wq_ca": g3[:, None] * wq_ca,
        "wk_ca": g2[:, None] * wk_ca,
        "wv_ca": g2[:, None] * wv_ca,
        "wp_ca": inp["Wp_ca"],
        "w1": g4[:, None] * inp["W1"],
        "w2": inp["W2"],
    }
    if with_bias:
        shared["ones_row"] = np.ones((1, 512), np.float32)
        for k, v in biases.items():
            shared[k] = v.reshape(1, C)
    shared = {k: np.ascontiguousarray(v.astype(BF)) for k, v in shared.items()}

    x = inp["x"]; xe = inp["x_encode"]
    in_maps = []
    for core in range(NCORES):
        b = core // 2
        half = core % 2
        own = slice(half * TOWN, (half + 1) * TOWN)
        other = slice((1 - half) * TOWN, (2 - half) * TOWN)
        # kv layout: [other half | own half]; own queries are rows 512..1023
        x_perm = np.concatenate([x[b, other, :], x[b, own, :]], axis=0)
        # other half: fully visible for half=1 (keys before queries), fully
        # hidden for half=0 -> v-flag 0/1; own half: shared triangle mask
        vf = np.zeros(T, np.float32)
        vf[:TOWN] = 1.0 if half == 1 else 0.0
        vf[TOWN:] = 1.0
        vf = np.ascontiguousarray(vf.reshape(NT_KV, 128).T)
        # readback row indices into the pair AllGather result: peer slice
        # first (kv layout is [other | own]); cols 0..7 per-hp K rows,
        # cols 8..11 per-tile V rows
        p128 = np.arange(128, dtype=np.int32)
        gi = np.zeros((128, 12), np.int32)
        for hp in range(NHP):
            gi[:, hp] = (1 - half) * (NHP * 128) + hp * 128 + p128
        for i in range(NT_OWN):
            gi[:, 8 + i] = (1 - half) * (NT_OWN * 128) + i * 128 + p128
        im = dict(shared)
        im["x_kv"] = np.ascontiguousarray(x_perm.astype(BF))
        im["xe"] = np.ascontiguousarray(
            np.concatenate([xe[b, other, :], xe[b, own, :]], axis=0).astype(BF))
        im["vflag"] = vf
        im["gidx"] = np.ascontiguousarray(gi)
        in_maps.append(im)
    return in_maps, with_bias


def run(inputs, trace=False, **kw):
    in_maps, with_bias = prepare_in_maps(inputs)
    nc = _get_nc(with_bias)
    r = bass_utils.run_bass_kernel_spmd(nc, in_maps, core_ids=list(range(NCORES)),
                                        trace=trace, **kw)
    out = np.empty((B, T, C), np.float32)
    for core in range(NCORES):
        b = core // 2
        half = core % 2
        out[b, half * TOWN:(half + 1) * TOWN, :] = r.results[core]["out"].astype(np.float32)
    return out, r


def kernel(**inputs):
    out, _ = run(inputs)
    return out


if __name__ == "__main__":
    build()
    print("build ok")

